# revision 1
# baseline (speedup 1.0000x reference)
"""Trainium2 Bass kernel for nn_AffineTransformLayer (B=8, C=4, H=W=1024).

Strategy (pure data parallel, batch element b -> NeuronCore b):
  1. Host computes, bit-exactly mirroring the jax-CPU reference, the per-pixel
     gather indices (i1, i0) and the four scalar blend weights per batch.
     (The reference's XLA-CPU einsum uses FMA contractions that cannot be
     reproduced bit-exactly by two-rounding device arithmetic; a 1-ulp index
     difference flips floor() and corrupts pixels, so indices ship from host.)
  2. Device premixes the 4 bilinear corners into one image V per channel
     (V[r,c] = w00*x[r,c] + w10*x[r,c+1] + w01*x[r+1,c] + w11*x[r+1,c+1]),
     so each output pixel becomes a single gather: out[y,x] = V[i1, i0].
  3. The 2D gather runs as 1024 tiles of 32x32 output pixels. Each tile gets
     a per-partition SBUF window = a dynamically anchored 184x148 box of V
     (fetched with a register-offset DMA) plus the 4 boundary lines of V
     (for clamped pixels outside the box). A GPSIMD ap_gather resolves the
     per-pixel window indices (host-packed int16 streams).
  4. Pixels whose sources fall outside box+lines (only possible for violently
     expansive transforms) are patched on host; for the benchmark inputs this
     is a tiny remainder and usually zero.
"""

import os
from contextlib import ExitStack

import numpy as np

H = W = 1024
C = 4
B = 8
TS = 32
NT = H // TS            # 32 tiles per side
NTILES = NT * NT        # 1024
NG = 8                  # ap_gather groups (Q7 cores)
NWAVES = NTILES // NG   # 128
HWIN, WWIN = 192, 148   # box dims (HWIN 8-aligned for the V_p8 layout)
NGRP8 = HWIN // 8       # 24 row-groups per box
BOX = HWIN * WWIN       # 28416
NLINE = 4 * H           # 4096
NE = BOX + NLINE        # 32512 window elements per partition
NIDX = TS * TS          # 1024 stream indices per tile
NBLK = H // 128         # 8 premix row blocks
# Per-group partition offset of the 4-channel window quad. Offsets chosen so
# each group's box-fetch DMA straddles an SDMA-engine boundary and the 8
# groups together cover all 16 engines (engine 2k+h serves partitions
# {64h + 4k + {0..3}, 64h + 32 + 4k + {0..3}}).
DQUAD = [2, 2, 10, 10, 2, 2, 10, 10]

_cache = {}


def _build_program():
    import concourse.bass as bass
    import concourse.bacc as bacc
    import concourse.tile as tile
    from concourse import mybir

    f32 = mybir.dt.float32
    i32 = mybir.dt.int32
    i16 = mybir.dt.int16
    Alu = mybir.AluOpType

    nc = bacc.Bacc("TRN2", target_bir_lowering=False, debug=False)
    x = nc.dram_tensor("x", [C, H, W], f32, kind="ExternalInput").ap()
    wts = nc.dram_tensor("wts", [1, 4], f32, kind="ExternalInput").ap()
    anc = nc.dram_tensor("anc", [1, NTILES], i32, kind="ExternalInput").ap()
    idxs = nc.dram_tensor("idxs", [NWAVES, 128, NIDX // 16], i16, kind="ExternalInput").ap()
    out = nc.dram_tensor("out", [C, H, W], f32, kind="ExternalOutput").ap()
    v = nc.dram_tensor("v", [C, H, W], f32).ap()
    vp8 = nc.dram_tensor("vp8", [C, H * W], f32).ap()
    ln = nc.dram_tensor("ln", [C, 4 * H], f32).ap()

    with tile.TileContext(nc) as tc, ExitStack() as ctx:
        cpool = ctx.enter_context(tc.tile_pool(name="const", bufs=1))

        wt = cpool.tile([128, 4], f32)
        nc.sync.dma_start(wt[:], wts[0:1, :].partition_broadcast(128))
        anct = cpool.tile([1, NTILES], i32)
        nc.sync.dma_start(anct[:], anc[:, :])

        # ---- premix: V = 4-corner blend of x ----
        with tc.tile_pool(name="pmx", bufs=2) as pmx:
            for blk in range(NBLK):
                y0 = blk * 128
                t0 = pmx.tile([128, C * W], f32, tag="t0")
                t0v = t0[:].rearrange("p (c n) -> p c n", c=C)
                nc.sync.dma_start(t0v, x[0:C, y0 : y0 + 128, :].transpose([1, 0, 2]))
                t1 = pmx.tile([128, C * W], f32, tag="t1")
                t1v = t1[:].rearrange("p (c n) -> p c n", c=C)
                if blk < NBLK - 1:
                    nc.sync.dma_start(t1v, x[0:C, y0 + 1 : y0 + 129, :].transpose([1, 0, 2]))
                else:
                    nc.vector.memset(t1[:], 0.0)
                    nc.sync.dma_start(
                        t1[0:127, :].rearrange("p (c n) -> p c n", c=C),
                        x[0:C, y0 + 1 : y0 + 128, :].transpose([1, 0, 2]),
                    )
                vo = pmx.tile([128, C * W], f32, tag="vo")
                vov = vo[:].rearrange("p (c n) -> p c n", c=C)
                for c in range(C):
                    ta = pmx.tile([128, W - 1], f32, tag="ta")
                    tb = pmx.tile([128, W - 1], f32, tag="tb")
                    nc.vector.tensor_scalar(
                        ta[:], t0v[:, c, 0 : W - 1], wt[:, 0:1], None, Alu.mult
                    )
                    nc.vector.scalar_tensor_tensor(
                        tb[:], t0v[:, c, 1:W], wt[:, 1:2], ta[:], Alu.mult, Alu.add
                    )
                    nc.vector.scalar_tensor_tensor(
                        ta[:], t1v[:, c, 0 : W - 1], wt[:, 2:3], tb[:], Alu.mult, Alu.add
                    )
                    nc.vector.scalar_tensor_tensor(
                        vov[:, c, 0 : W - 1], t1v[:, c, 1:W], wt[:, 3:4], ta[:], Alu.mult, Alu.add
                    )
                    nc.vector.tensor_copy(vov[:, c, W - 1 : W], t0v[:, c, W - 1 : W])
                nc.sync.dma_start(v[0:C, y0 : y0 + 128, :].transpose([1, 0, 2]), vov)

        # ---- boundary lines of V: [left col, right col, top row, bottom row] ----
        with nc.allow_non_contiguous_dma(reason="column line extraction"):
            for c in range(C):
                nc.scalar.dma_start(ln[c, 0:H], v[c, 0:H, 0:1].rearrange("r o -> (r o)"))
                nc.scalar.dma_start(
                    ln[c, H : 2 * H], v[c, 0:H, W - 2 : W - 1].rearrange("r o -> (r o)")
                )
                nc.scalar.dma_start(ln[c, 2 * H : 3 * H], v[c, 0, :])
                nc.scalar.dma_start(ln[c, 3 * H : 4 * H], v[c, H - 2, :])

        # ---- reformat V into 8-row-interleaved V_p8: [rowgroup][col][parity] ----
        # A box fetch from V_p8 needs one contiguous 148*8*4B descriptor per
        # row-group instead of 8 x 592B row descriptors.
        with tc.tile_pool(name="rfmt", bufs=2) as rf:
            for c in range(C):
                rt = rf.tile([128, 8 * W], f32, tag="rt")
                nc.sync.dma_start(
                    rt[:], v[c, :, :].rearrange("(p a) b -> p (a b)", a=8)
                )
                it2 = rf.tile([128, 8 * W], f32, tag="it2")
                nc.vector.tensor_copy(
                    it2[:].rearrange("p (b a) -> p b a", a=8),
                    rt[:].rearrange("p (a b) -> p a b", a=8).transpose([0, 2, 1]),
                )
                nc.sync.dma_start(
                    vp8[c, :].rearrange("(p n) -> p n", p=128), it2[:]
                )

        # ---- gather waves ----
        gpool = ctx.enter_context(tc.tile_pool(name="gather", bufs=1))
        win = gpool.tile([128, NE], f32)
        nc.vector.memset(win[:], 0.0)
        for g in range(NG):
            d = DQUAD[g]
            nc.scalar.dma_start(win[16 * g + d : 16 * g + d + C, BOX:NE], ln[0:C, :])

        _engs3 = [nc.sync, nc.scalar, nc.gpsimd]
        box_engs = [_engs3[g % 3] for g in range(NG)]
        regs = [box_engs[g].alloc_register(f"boxoff{g}") for g in range(NG)]

        with tc.tile_pool(name="wave", bufs=2) as wpool:
            for wv in range(NWAVES):
                it = wpool.tile([128, NIDX // 16], i16, tag="it")
                nc.sync.dma_start(it[:], idxs[wv, :, :])
                for g in range(NG):
                    t = wv * NG + g
                    eng = box_engs[g]
                    eng.reg_load(regs[g], anct[0:1, t : t + 1])
                    src = bass.AP(
                        vp8.tensor, regs[g], [[H * W, C], [8 * W, NGRP8], [1, WWIN * 8]]
                    )
                    d = DQUAD[g]
                    eng.dma_start(
                        win[16 * g + d : 16 * g + d + C, 0:BOX].rearrange(
                            "p (r c2) -> p r c2", r=NGRP8
                        ),
                        src,
                    )
                go = wpool.tile([128, NIDX], f32, tag="go")
                nc.gpsimd.ap_gather(
                    go[:], win[:], it[:], channels=128, num_elems=NE, d=1, num_idxs=NIDX
                )
                for g in range(NG):
                    t = wv * NG + g
                    ty, tx = divmod(t, NT)
                    d = DQUAD[g]
                    oeng = nc.gpsimd
                    oeng.dma_start(
                        out[0:C, ty * TS : (ty + 1) * TS, tx * TS : (tx + 1) * TS],
                        go[16 * g + d : 16 * g + d + C, :].rearrange(
                            "p (a b) -> p a b", a=TS
                        ),
                    )

    nc.compile()
    return nc


def _plan(x_np, transform_np):
    """Host planner: bit-exact indices/weights (mirrors jax-CPU reference),
    per-core tile anchors, int16 gather streams, and host-patch values."""
    import jax
    import jax.numpy as jnp

    cpu = jax.devices("cpu")[0]
    with jax.default_device(cpu):
        transform = jnp.asarray(transform_np)
        A = transform[:, :4].reshape(B, 2, 2)
        t = transform[:, 4:6].reshape(B, 1, 2)
        Ainv = jnp.linalg.inv(A)
        t_inv = -jnp.matmul(t, Ainv)
        xg, yg = jnp.meshgrid(jnp.arange(W), jnp.arange(H), indexing="ij")
        pix = jnp.stack([xg.ravel(), yg.ravel()], axis=-1).astype(jnp.float32)
        out_pix = jnp.einsum("ni,bij->bnj", pix, Ainv) + t_inv
        c0 = jnp.clip(out_pix[..., 0], 0.0, H - 2)
        c1 = jnp.clip(out_pix[..., 1], 0.0, W - 2)
        i0 = c0.astype(jnp.int32)
        i1 = c1.astype(jnp.int32)
        dx0 = (c0 - i0.astype(jnp.float32))[:, 0]
        dy0 = (c1 - i1.astype(jnp.float32))[:, 0]
        i0 = np.asarray(i0)
        i1 = np.asarray(i1)
        dx0 = np.asarray(dx0)
        dy0 = np.asarray(dy0)

    plans = []
    for b in range(B):
        # row-major [y, x] index maps (k = x*H + y in reference order)
        I0 = np.ascontiguousarray(i0[b].reshape(W, H).T)
        I1 = np.ascontiguousarray(i1[b].reshape(W, H).T)
        w00 = np.float32((1 - dx0[b]) * (1 - dy0[b]))
        w10 = np.float32(dx0[b] * (1 - dy0[b]))
        w01 = np.float32((1 - dx0[b]) * dy0[b])
        w11 = np.float32(dx0[b] * dy0[b])
        wts = np.array([[w00, w10, w01, w11]], dtype=np.float32)

        # tiles [ty, tx, yl, xl]
        I0t = I0.reshape(NT, TS, NT, TS).transpose(0, 2, 1, 3)
        I1t = I1.reshape(NT, TS, NT, TS).transpose(0, 2, 1, 3)
        R0 = np.minimum(I1t.min(axis=(2, 3), keepdims=True), H - HWIN) & ~7
        C0 = np.minimum(I0t.min(axis=(2, 3), keepdims=True), W - WWIN)
        inbox = (
            (I1t >= R0)
            & (I1t < R0 + HWIN)
            & (I0t >= C0)
            & (I0t < C0 + WWIN)
        )
        dr = I1t - R0
        widx = (dr >> 3) * (WWIN * 8) + (I0t - C0) * 8 + (dr & 7)
        widx = np.where(inbox, widx, 0)
        lined = np.zeros_like(inbox)
        for cond, off, val in (
            (I0t == 0, BOX, I1t),
            (I0t == W - 2, BOX + H, I1t),
            (I1t == 0, BOX + 2 * H, I0t),
            (I1t == H - 2, BOX + 3 * H, I0t),
        ):
            use = cond & ~inbox & ~lined
            widx = np.where(use, off + val, widx)
            lined |= use
        covered = inbox | lined
        patch = ~covered

        anchors = (
            ((R0[:, :, 0, 0] >> 3) * (8 * W) + C0[:, :, 0, 0] * 8)
            .astype(np.int32)
            .reshape(1, NTILES)
        )
        # streams: tile t = wave*8 + g; stream pos j = yl*TS + xl;
        # wrapped: idxs[wave, 16g + j%16, j//16]
        wtile = widx.reshape(NTILES, NIDX).astype(np.int16)  # [t, j]
        wtile = wtile.reshape(NWAVES, NG, NIDX // 16, 16)  # [wave, g, s, j%16]
        streams = np.ascontiguousarray(
            wtile.transpose(0, 1, 3, 2).reshape(NWAVES, 128, NIDX // 16)
        )

        # host patch values (premixed blend, same f32 op order as device)
        py, px_ = np.nonzero(patch.transpose(0, 2, 1, 3).reshape(H, W))
        pv = None
        if len(py):
            r = I1.reshape(H, W)[py, px_]
            c = I0.reshape(H, W)[py, px_]
            xb = x_np[b]  # [C, H, W]
            pv = (
                (xb[:, r, c] * w00 + xb[:, r, c + 1] * w10)
                + xb[:, r + 1, c] * w01
            ) + xb[:, r + 1, c + 1] * w11  # [C, npatch]
        plans.append(
            dict(
                wts=wts,
                anc=anchors,
                idxs=streams,
                patch_yx=(py, px_),
                patch_vals=pv,
            )
        )
    return plans


def kernel(x, transform):
    """x: [8, 4, 1024, 1024] f32; transform: [8, 6] f32 -> [8, 4, 1024, 1024] f32."""
    from concourse.bass_utils import run_bass_kernel_spmd

    x = np.asarray(x, dtype=np.float32)
    transform = np.asarray(transform, dtype=np.float32)

    if "nc" not in _cache:
        _cache["nc"] = _build_program()
    nc = _cache["nc"]

    plans = _plan(x, transform)
    in_maps = []
    for b in range(B):
        p = plans[b]
        in_maps.append(
            {"x": x[b], "wts": p["wts"], "anc": p["anc"], "idxs": p["idxs"]}
        )
    res = run_bass_kernel_spmd(nc, in_maps, list(range(B)))
    outs = []
    for b in range(B):
        ob = res.results[b]["out"]
        py, px_ = plans[b]["patch_yx"]
        if len(py):
            ob = ob.copy()
            ob[:, py, px_] = plans[b]["patch_vals"]
        outs.append(ob)
    return np.stack(outs).astype(np.float32)



# revision 15
# speedup vs baseline: 2.1911x; 2.1911x over previous
"""Trainium2 Bass kernel for nn_AffineTransformLayer (B=8, C=4, H=W=1024).

Panel-gather design (pure data parallel, batch b -> NeuronCore b):
  1. Host computes per-pixel gather indices bit-matching the reference's
     f32 index math; values flow in bf16 (global tolerance 2e-2).
  2. Device premixes the 4 bilinear corners into one image V per channel
     (weights are per-batch scalars), stored as overlapping column panels
     P[cp][pan][rowgroup][col][row%8][c01] in bf16 so any 32x32 output
     tile's source window is ONE contiguous DRAM run.
  3. Windows for 8 waves (64 tiles) are fetched by a single table-driven
     indirect DMA into all 128 partitions (slot p = 16*tile + 2*(w%8)+cp).
     Clamped pixels resolve against a host-shipped boundary-lines block
     appended to the window region.
  4. One ap_gather (d=2 channel pairs) per 8-tile wave; a DVE
     de-interleave + two partition-permute SBUF->SBUF DMAs assemble
     32-row output strips; a fused 32-block stream-transpose writes
     f32 strips out with 4KB descriptors.
  5. Tiles whose window exceeds the static panel/rowgroup caps (~1% of
     pixels, extreme transforms) are patched on host.
"""

from contextlib import ExitStack

import numpy as np
import ml_dtypes

bf16 = ml_dtypes.bfloat16

H = W = 1024
C = 4
B = 8
TS = 32
NT = H // TS              # 32 tiles per side
TPW = 8                   # tiles per wave (one per Q7 core)
NW = NT * NT // TPW       # 128 waves
NGRP = 8                  # waves per indirect fetch group
NGROUPS = NW // NGRP      # 16
PW = 80                   # panel width (cols)
PST = 32                  # panel stride
NPAN = (W - PW + PST - 1) // PST + 1   # 31
NG8CAP = 16               # max rowgroups per window
RUNMAX = NG8CAP * PW * 8  # du (pixel slots) per window buf: 10240
LINES = 4 * H             # 4096 du of line pixels
NE_G = RUNMAX + LINES     # gather num_elems (du): 14336
PANELEMS = PW * 16        # bf16 elems per (pan, k) slab: 1280
PCPBASE = NPAN * 128 * PANELEMS
PELEMS = 2 * PCPBASE
PPAD = 2 * RUNMAX

_cache = {}


def _pan_start(pan):
    return min(pan * PST, W - PW)


def _wave_coords(w):
    sy = w // 4
    txs = [(w % 4) * 8 + ti for ti in range(TPW)]
    return sy, txs


def _build_program(group_ng8, external_panels=False):
    import concourse.bass as bass
    import concourse.bacc as bacc
    import concourse.tile as tile
    from concourse import mybir

    f32 = mybir.dt.float32
    i32 = mybir.dt.int32
    i16 = mybir.dt.int16
    bf = mybir.dt.bfloat16
    Alu = mybir.AluOpType

    nc = bacc.Bacc("TRN2", target_bir_lowering=False, debug=False)
    xp = nc.dram_tensor("xp", [C, H + 1, W], f32, kind="ExternalInput").ap()
    wts = nc.dram_tensor("wts", [1, 4], f32, kind="ExternalInput").ap()
    lnt = nc.dram_tensor("lnt", [128, 2 * LINES], bf, kind="ExternalInput").ap()
    idxt = nc.dram_tensor("idxt", [128, NW * 64], i16, kind="ExternalInput").ap()
    tabt = nc.dram_tensor("tabt", [128, NGROUPS], i32, kind="ExternalInput").ap()
    out = nc.dram_tensor("out", [C, H, W], f32, kind="ExternalOutput").ap()
    Pt = nc.dram_tensor(
        "Pt", [PELEMS + PPAD], bf,
        kind="ExternalInput" if external_panels else "Internal",
    ).ap()

    # window region element offsets (bf16 elems)
    BUFA = 0
    LINE0 = 2 * RUNMAX
    BUFB = 2 * RUNMAX + 2 * LINES
    WINE = 2 * (2 * RUNMAX + LINES)   # 49152 elems

    with tile.TileContext(nc) as tc, ExitStack() as ctx:
        cpool = ctx.enter_context(tc.tile_pool(name="const", bufs=1))
        wt = cpool.tile([128, 4], f32)
        nc.sync.dma_start(wt[:], wts[0:1, :].partition_broadcast(128))

        gpool = ctx.enter_context(tc.tile_pool(name="gat", bufs=1))
        # win doubles as premix scratch: xsb (73.7KB) + vf (32KB) need 53248 elems
        win = gpool.tile([128, max(WINE, 53248)], bf)
        pb = gpool.tile([128, W * 8 * 2], bf)
        idxsb = gpool.tile([128, NW * 64], i16)
        tabsb = gpool.tile([128, NGROUPS], i32)
        nc.sync.dma_start(idxsb[:], idxt)
        nc.sync.dma_start(tabsb[:], tabt)

        # ---------------- premix into panels ----------------
        # scratch views inside win (reused before gather phase starts)
        xsb = win[:, 0:2 * 2 * 9 * W].bitcast(f32)     # [128, 9216] f32
        vfa = win[:, 2 * 2 * 9 * W: 2 * 2 * 9 * W + 2 * 8 * W].bitcast(f32)
        for cp in ([] if external_panels else range(2)):
            src = bass.AP(
                xp.tensor,
                2 * cp * ((H + 1) * W),
                [[8 * W, 128], [(H + 1) * W, 2], [W, 9], [1, W]],
            )
            nc.sync.dma_start(
                xsb.rearrange("p (c r e) -> p c r e", c=2, r=9), src
            )
            xv = xsb.rearrange("p (c r e) -> p c r e", c=2, r=9)
            vv = vfa.rearrange("p (r e) -> p r e", r=8)
            pbv = pb[:].rearrange("p (e r c) -> p e r c", e=W, r=8)
            for c2 in range(2):
                a = xv[:, c2, 0:8, 0:W - 1]
                bb = xv[:, c2, 0:8, 1:W]
                d_ = xv[:, c2, 1:9, 0:W - 1]
                e_ = xv[:, c2, 1:9, 1:W]
                o = vv[:, :, 0:W - 1]
                nc.vector.tensor_scalar(o, a, wt[:, 0:1], None, Alu.mult)
                nc.vector.scalar_tensor_tensor(o, bb, wt[:, 1:2], o, Alu.mult, Alu.add)
                nc.vector.scalar_tensor_tensor(o, d_, wt[:, 2:3], o, Alu.mult, Alu.add)
                # final op writes transposed+cast directly into pb[:, :, :, c2]
                nc.vector.scalar_tensor_tensor(
                    pbv[:, 0:W - 1, :, c2].transpose([0, 2, 1]),
                    e_, wt[:, 3:4], o, Alu.mult, Alu.add,
                )
                nc.vector.memset(pbv[:, W - 1:W, :, c2], 0.0)
            # panels 0..29 (uniform stride PST*16 elems), pan 30 separate
            pbap = pb[:]
            src_pan = bass.AP(
                pbap.tensor, pbap.offset,
                [pbap.ap[0], [PST * 16, NPAN - 1], [1, PANELEMS]],
            )
            dst_pan = bass.AP(
                Pt.tensor, cp * PCPBASE,
                [[PANELEMS, 128], [128 * PANELEMS, NPAN - 1], [1, PANELEMS]],
            )
            nc.scalar.dma_start(dst_pan, src_pan)
            lastoff = _pan_start(NPAN - 1) * 16
            dst_last = bass.AP(
                Pt.tensor, cp * PCPBASE + (NPAN - 1) * 128 * PANELEMS,
                [[PANELEMS, 128], [1, PANELEMS]],
            )
            nc.scalar.dma_start(dst_last, pb[:, lastoff:lastoff + PANELEMS])

        # ---------------- gather phase ----------------
        if not external_panels:
            # init Pt pad (group-fetch overhang may read it)
            nc.vector.memset(pb[:, 0:PPAD // 128], 0.0)
            nc.sync.dma_start(
                bass.AP(Pt.tensor, PELEMS, [[PPAD // 128, 128], [1, PPAD // 128]]),
                pb[:, 0:PPAD // 128],
            )
        nc.vector.memset(win[:], 0.0)
        # lines: per-partition variant shipped from host
        nc.sync.dma_start(win[:, LINE0:LINE0 + 2 * LINES], lnt)

        ptv = Pt.rearrange("(n o) -> n o", o=1)
        Bcur = None
        with tc.tile_pool(name="wv", bufs=2) as wpool, \
                tc.tile_pool(name="st", bufs=2) as spool:
            for g in range(NGROUPS):
                run8 = group_ng8[g] * PW * 8      # du
                par = g % 2
                base = BUFA if par == 0 else BUFB
                nc.gpsimd.indirect_dma_start(
                    out=win[:, base:base + 2 * run8],
                    out_offset=None,
                    in_=ptv,
                    in_offset=bass.IndirectOffsetOnAxis(ap=tabsb[:, g:g + 1], axis=0),
                )
                inap = (win[:, 0:2 * NE_G] if par == 0
                        else win[:, LINE0:LINE0 + 2 * NE_G])
                for w8 in range(NGRP):
                    w = g * NGRP + w8
                    sy = w // 4
                    if w % 4 == 0:
                        Bcur = spool.tile([128, TS * TS], bf, tag="B")
                    gout = wpool.tile([128, 2 * TS * TS], bf, tag="gout")
                    nc.gpsimd.ap_gather(
                        gout[:], inap, idxsb[:, w * 64:(w + 1) * 64],
                        channels=128, num_elems=NE_G, d=2, num_idxs=TS * TS,
                    )
                    gd = wpool.tile([128, 2 * TS * TS], bf, tag="gd")
                    gdv = gd[:].rearrange("p (c e) -> p c e", c=2)
                    gov = gout[:].rearrange("p (e c) -> p e c", c=2).transpose([0, 2, 1])
                    if w % 2 == 0:
                        nc.vector.tensor_copy(gdv, gov)
                    else:
                        nc.scalar.copy(gdv, gov)
                    # partition-permute SBUF->SBUF into strip accumulator
                    # (single strided partition dim per AP: split by cp, c01)
                    gsrc = gd[:].rearrange("(ti q) e -> ti q e", q=16)
                    bdst = Bcur[:].rearrange("(cc t32) e -> cc t32 e", t32=32)
                    for cp in range(2):
                        for c01 in range(2):
                            seng = nc.sync if c01 == 0 else nc.scalar
                            seng.dma_start(
                                bdst[2 * cp + c01,
                                     8 * (w % 4):8 * (w % 4) + 8, :],
                                gsrc[:, 2 * w8 + cp,
                                     c01 * TS * TS:(c01 + 1) * TS * TS],
                            )
                    if w % 4 == 3:
                        bp = spool.tile([128, TS * TS], bf, tag="bp")
                        nc.vector.transpose(bp[:], Bcur[:])
                        D = spool.tile([128, TS * TS], f32, tag="D")
                        nc.scalar.copy(
                            D[:].rearrange("p (t xl) -> p t xl", t=TS),
                            bp[:].rearrange("p (xl t) -> p xl t", xl=TS)
                                 .transpose([0, 2, 1]),
                        )
                        oeng = nc.sync if (sy % 2 == 0) else nc.scalar
                        oeng.dma_start(
                            out[0:C, sy * TS:(sy + 1) * TS, :],
                            D[:].rearrange("p (a b) -> p a b", a=TS),
                        )

    nc.compile()
    return nc


def _plan(x, transform):
    """Host planner. Returns (in_maps, patches, group_ng8)."""
    import jax
    import jax.numpy as jnp

    cpu = jax.devices("cpu")[0]
    with jax.default_device(cpu):
        tr = jnp.asarray(transform)
        A = tr[:, :4].reshape(B, 2, 2)
        t = tr[:, 4:6].reshape(B, 1, 2)
        Ainv = jnp.linalg.inv(A)
        t_inv = -jnp.matmul(t, Ainv)
        xg, yg = jnp.meshgrid(jnp.arange(W), jnp.arange(H), indexing="ij")
        pix = jnp.stack([xg.ravel(), yg.ravel()], -1).astype(jnp.float32)
        out_pix = jnp.einsum("ni,bij->bnj", pix, Ainv) + t_inv
        c0r = np.asarray(out_pix[..., 0])
        c1r = np.asarray(out_pix[..., 1])
    c0 = np.clip(c0r, 0.0, H - 2)
    c1 = np.clip(c1r, 0.0, W - 2)
    i0 = c0.astype(np.int32)
    i1 = c1.astype(np.int32)
    dx0 = (c0 - i0)[:, 0]
    dy0 = (c1 - i1)[:, 0]
    bmk = (c0r >= 0) & (c0r <= H - 2) & (c1r >= 0) & (c1r <= W - 2)

    cores = []
    for b in range(B):
        I0 = np.ascontiguousarray(i0[b].reshape(W, H).T)
        I1 = np.ascontiguousarray(i1[b].reshape(W, H).T)
        M = np.ascontiguousarray(bmk[b].reshape(W, H).T)
        I0t = I0.reshape(NT, TS, NT, TS).transpose(0, 2, 1, 3)
        I1t = I1.reshape(NT, TS, NT, TS).transpose(0, 2, 1, 3)
        Mt = M.reshape(NT, TS, NT, TS).transpose(0, 2, 1, 3)
        pan_t = np.zeros((NT, NT), np.int32)
        k0_t = np.zeros((NT, NT), np.int32)
        ng8_t = np.zeros((NT, NT), np.int32)
        fit_t = np.zeros((NT, NT), bool)
        for ty in range(NT):
            for tx in range(NT):
                m = Mt[ty, tx]
                if not m.any():
                    continue
                r = I1t[ty, tx][m]
                c = I0t[ty, tx][m]
                k0 = int(r.min()) >> 3
                ng8 = (int(r.max()) >> 3) - k0 + 1
                cmin, cmax = int(c.min()), int(c.max())
                hi = min(cmin // PST, NPAN - 1)
                pan = hi
                fits = (ng8 <= NG8CAP) and (cmax < _pan_start(pan) + PW)
                if (not fits and hi < NPAN - 1 and _pan_start(hi + 1) <= cmin
                        and cmax < _pan_start(hi + 1) + PW and ng8 <= NG8CAP):
                    pan = hi + 1
                    fits = True
                pan_t[ty, tx] = pan
                k0_t[ty, tx] = k0
                ng8_t[ty, tx] = ng8
                fit_t[ty, tx] = fits
        cores.append(dict(pan=pan_t, k0=k0_t, ng8=ng8_t, fit=fit_t,
                          I0t=I0t, I1t=I1t, Mt=Mt, I0=I0, I1=I1,
                          dx0=np.float32(dx0[b]), dy0=np.float32(dy0[b])))

    group_ng8 = []
    for g in range(NGROUPS):
        mx = 1
        for w in range(g * NGRP, (g + 1) * NGRP):
            sy, txs = _wave_coords(w)
            for pc in cores:
                for tx in txs:
                    if pc["fit"][sy, tx]:
                        mx = max(mx, int(pc["ng8"][sy, tx]))
        group_ng8.append(mx)

    in_maps = []
    patches = []
    for b in range(B):
        pc = cores[b]
        dxb, dyb = pc["dx0"], pc["dy0"]
        w00 = np.float32((1 - dxb) * (1 - dyb))
        w10 = np.float32(dxb * (1 - dyb))
        w01 = np.float32((1 - dxb) * dyb)
        w11 = np.float32(dxb * dyb)
        wtsb = np.array([[w00, w10, w01, w11]], np.float32)

        # lines from f32 premix of edges only
        xb = x[b].astype(np.float32)
        Vc0 = ((xb[:, :H - 1, 0] * w00 + xb[:, :H - 1, 1] * w10)
               + xb[:, 1:, 0] * w01) + xb[:, 1:, 1] * w11           # col 0
        Vc1 = ((xb[:, :H - 1, W - 2] * w00 + xb[:, :H - 1, W - 1] * w10)
               + xb[:, 1:, W - 2] * w01) + xb[:, 1:, W - 1] * w11   # col 1022
        Vr0 = ((xb[:, 0, :W - 1] * w00 + xb[:, 0, 1:] * w10)
               + xb[:, 1, :W - 1] * w01) + xb[:, 1, 1:] * w11       # row 0
        Vr1 = ((xb[:, H - 2, :W - 1] * w00 + xb[:, H - 2, 1:] * w10)
               + xb[:, H - 1, :W - 1] * w01) + xb[:, H - 1, 1:] * w11
        ln = np.zeros((2, 4, H, 2), bf16)
        for cp in range(2):
            for c01 in range(2):
                ch = 2 * cp + c01
                ln[cp, 0, :H - 1, c01] = Vc0[ch]
                ln[cp, 1, :H - 1, c01] = Vc1[ch]
                ln[cp, 2, :W - 1, c01] = Vr0[ch]
                ln[cp, 3, :W - 1, c01] = Vr1[ch]

        idx = np.zeros((128, NW * 64), np.int16)
        tab = np.zeros((128, NGROUPS), np.int32)
        patch_y = []
        patch_x = []
        for w in range(NW):
            sy, txs = _wave_coords(w)
            g = w // NGRP
            par = g % 2
            base_box = 0 if par == 0 else LINES
            base_line = RUNMAX if par == 0 else 0
            for ti, tx in enumerate(txs):
                fits = bool(pc["fit"][sy, tx])
                pan = int(pc["pan"][sy, tx])
                k0 = int(pc["k0"][sy, tx])
                m = pc["Mt"][sy, tx]
                r = pc["I1t"][sy, tx].astype(np.int64)
                c = pc["I0t"][sy, tx].astype(np.int64)
                isbox = m & fits
                du = ((r >> 3) - k0) * (PW * 8) + (c - _pan_start(pan)) * 8 + (r & 7)
                e = np.where(isbox, base_box + du, 0)
                notbox = ~m
                cnd0 = notbox & (c == 0)
                cnd1 = notbox & (c == W - 2) & ~cnd0
                cnd2 = notbox & (r == 0) & ~cnd0 & ~cnd1
                cnd3 = notbox & (r == H - 2) & ~cnd0 & ~cnd1 & ~cnd2
                lidx = np.select([cnd0, cnd1, cnd2, cnd3], [0, 1, 2, 3], 0)
                lpx = np.select([cnd0, cnd1, cnd2, cnd3], [r, r, c, c], 0)
                e = np.where(notbox, base_line + lidx * H + lpx, e)
                if not fits and m.any():
                    yy, xx = np.nonzero(m)
                    patch_y.append(sy * TS + yy)
                    patch_x.append(tx * TS + xx)
                stream = e.T.reshape(TS * TS)
                wrapped = stream.reshape(64, 16).T
                idx[16 * ti:16 * ti + 16, w * 64:(w + 1) * 64] = wrapped.astype(np.int16)
                for cp in range(2):
                    p = 16 * ti + 2 * (w % NGRP) + cp
                    if fits:
                        tab[p, g] = (cp * PCPBASE + pan * (128 * PANELEMS)
                                     + k0 * PANELEMS)
        if patch_y:
            py = np.concatenate(patch_y)
            px_ = np.concatenate(patch_x)
            rr = pc["I1"][py, px_].astype(np.int64)
            cc = pc["I0"][py, px_].astype(np.int64)
            pv = (((xb[:, rr, cc] * w00 + xb[:, rr, cc + 1] * w10)
                   + xb[:, rr + 1, cc] * w01) + xb[:, rr + 1, cc + 1] * w11)
        else:
            py = np.zeros(0, np.int64)
            px_ = np.zeros(0, np.int64)
            pv = None
        patches.append((py, px_, pv))

        xpad = np.zeros((C, H + 1, W), np.float32)
        xpad[:, :H, :] = x[b]
        lnflat = ln.reshape(2, 2 * LINES)
        lnfull = np.zeros((128, 2 * LINES), bf16)
        lnfull[0::2] = lnflat[0]
        lnfull[1::2] = lnflat[1]
        in_maps.append({
            "xp": xpad,
            "wts": wtsb,
            "lnt": lnfull,
            "idxt": idx,
            "tabt": tab,
        })
    return in_maps, patches, group_ng8


def kernel(x, transform):
    """x: [8, 4, 1024, 1024] f32; transform: [8, 6] f32 -> [8, 4, 1024, 1024] f32."""
    from concourse.bass_utils import run_bass_kernel_spmd

    x = np.asarray(x, dtype=np.float32)
    transform = np.asarray(transform, dtype=np.float32)

    in_maps, patches, group_ng8 = _plan(x, transform)
    key = tuple(group_ng8)
    if key not in _cache:
        _cache[key] = _build_program(group_ng8)
    nc = _cache[key]

    res = run_bass_kernel_spmd(nc, in_maps, list(range(B)))
    outs = []
    for b in range(B):
        ob = res.results[b]["out"]
        py, px_, pv = patches[b]
        if len(py):
            ob = ob.copy()
            ob[:, py, px_] = pv
        outs.append(ob)
    return np.stack(outs).astype(np.float32)
